# revision 2
# baseline (speedup 1.0000x reference)
"""Trainium2 Bass kernel for nn_CrossAttention (sparse_attention) — v3.

Sharding: data-parallel over B across 8 NeuronCores (1 batch element per
core, weights replicated, no collectives).

Design (vs 553us baseline):
  - feature-major x-path: q computed transposed (Wq^T @ xln^T) from
    DMA-transposed x_ln loads; head-softmax denominators via mask-matmul
    accumulation; per-head normalization via replication-matmuls.
    No PE transposes in the x-path at all.
  - every LN rstd = exp(-0.5*ln(var+eps)): ln+exp live in the SAME ACT
    table set, so rstds mix freely with the kv/q exps -> only the
    exp<->silu superbatch switches remain (~9 table loads total).
  - x prepass (bf16 stats + LN apply + x_ln store) emitted in per-group
    chunks interleaved into the n/s kv loops: the in-order DVE queue
    never waits on unlanded x DMA, and x_pre pool slots recycle fast.
  - x-phase in 4-group superbatches ([A x4][B x4]) with the A stages
    software-pipelined across groups (qproj g+1 between dsum/rep g).
"""
import numpy as np

H, D, TFD, AUD, EPS = 16, 1024, 256, 768, 1e-5
B, T, N, S = 8, 4096, 512, 512
dh = D // H
P = 128
TT = T // P           # 32 token tiles
NT = 2 * N // P       # 8 n tiles
ST = S // P           # 4 s tiles
DC = D // P           # 8 feature chunks
NG = TT // 4          # 8 token groups of 512
SB = 4                # groups per superbatch (ACT table batching)
NCORES = 8

_CACHE = {}


def _build(affine_x, affine_t, affine_s, hasb=None, dbg=False):
    import concourse.bass as bass
    import concourse.tile as tile
    from concourse import bacc, mybir
    from concourse.masks import make_identity

    if hasb is None:
        hasb = {}
    FP32 = mybir.dt.float32
    BF16 = mybir.dt.bfloat16
    AF = mybir.ActivationFunctionType
    OP = mybir.AluOpType

    nc = bacc.Bacc()

    # ---------------- DRAM parameters (per-core shapes) ----------------
    x_ext = nc.declare_dram_parameter("x", [T, D], FP32, isOutput=False)
    xf_ext = nc.declare_dram_parameter("xf", [AUD], FP32, isOutput=False)
    xw_ext = nc.declare_dram_parameter("xw", [N, TFD], FP32, isOutput=False)
    xs_ext = nc.declare_dram_parameter("xs", [S, D], FP32, isOutput=False)
    wext = {}
    for nm, shp in [
        ("norm_g", [D]), ("norm_b", [D]), ("tnorm_g", [D]), ("tnorm_b", [D]),
        ("snorm_g", [D]), ("snorm_b", [D]),
        ("Wq", [D, D]), ("bq", [D]), ("Wk", [D, D]), ("bk", [D]),
        ("Wv", [D, D]), ("bv", [D]), ("Wa", [AUD, TFD]), ("ba", [TFD]),
        ("Wat", [TFD, D]), ("bat", [D]), ("Wo", [D, D]), ("bo", [D]),
    ]:
        wext[nm] = nc.declare_dram_parameter(nm, shp, FP32, isOutput=False)
    out_ext = nc.declare_dram_parameter("out", [T, D], FP32, isOutput=True)
    # per-group x_ln staging in DRAM (bf16, token-major); per-group tensors
    # so the transposed reloads only depend on their own group's stores
    xln_dram = [nc.dram_tensor(f"x_ln{g}", [4 * P, D], BF16)
                for g in range(NG)]

    with tile.TileContext(nc) as tc, \
         tc.tile_pool(name="wpool", bufs=1) as wpool, \
         tc.tile_pool(name="npool", bufs=1) as npool, \
         tc.tile_pool(name="work", bufs=2) as work, \
         tc.tile_pool(name="xtp", bufs=2) as xtp, \
         tc.tile_pool(name="pproj", bufs=2, space="PSUM") as pproj:

        # ---------------- constants ----------------
        ident_bf = wpool.tile([P, P], BF16, tag="ident_bf")
        make_identity(nc, ident_bf)
        ones1_bf = wpool.tile([1, P], BF16, tag="ones1_bf")
        nc.vector.memset(ones1_bf, 1.0)
        ones1_f = wpool.tile([1, P], FP32, tag="ones1_f")
        nc.vector.memset(ones1_f, 1.0)
        onescol_bf = wpool.tile([P, 1], BF16, tag="onescol_bf")
        nc.vector.memset(onescol_bf, 1.0)
        onescol_f = wpool.tile([P, 1], FP32, tag="onescol_f")
        nc.vector.memset(onescol_f, 1.0)
        mask_f = wpool.tile([P, P], FP32, tag="mask_f")
        nc.vector.memset(mask_f, 0.0)
        nc.vector.memset(mask_f[0:dh, 0:dh], 1.0)
        nc.vector.memset(mask_f[dh:P, dh:P], 1.0)
        eps_t = wpool.tile([P, 1], FP32, tag="eps_t")
        nc.vector.memset(eps_t, EPS)
        ln512_t = wpool.tile([1, 1], FP32, tag="ln512_t")
        nc.vector.memset(ln512_t, float(np.log(N)))
        # hmask[p, c, h] = 1 if head h covers partition p in chunk c
        hmask = wpool.tile([P, DC, H], BF16, tag="hmask")
        nc.vector.memset(hmask, 0.0)
        for c in range(DC):
            nc.vector.memset(hmask[0:dh, c, 2 * c:2 * c + 1], 1.0)
            nc.vector.memset(hmask[dh:P, c, 2 * c + 1:2 * c + 2], 1.0)
        # hsel[h, c, p] = hmask[p, c, h]; built via PE transposes below
        hsel = wpool.tile([H, DC, P], BF16, tag="hsel")
        ones512_bf = None
        if any(hasb.get(k, True) for k in ("bq",)):
            ones512_bf = wpool.tile([1, 512], BF16, tag="ones512_bf")
            nc.vector.memset(ones512_bf, 1.0)

        # ---------------- weights (casting DMAs are gpsimd-only) --------
        def load_w(nm, rows, cols, pool=None):
            t = (pool or wpool).tile([P, rows // P, cols], BF16, tag=nm)
            src = wext[nm][:, :].rearrange("(c p) n -> p c n", p=P)
            nc.gpsimd.dma_start(out=t, in_=src)
            return t

        def load_row(nm, L):
            if not hasb.get(nm, True):
                return None
            t = wpool.tile([1, L], BF16, tag=nm + "_r")
            nc.gpsimd.dma_start(out=t, in_=wext[nm][:][None, :])
            return t

        xf_col = wpool.tile([P, AUD // P], BF16, tag="xf_col")
        nc.gpsimd.dma_start(out=xf_col,
                            in_=xf_ext[:].rearrange("(c p) -> p c", p=P))
        Wa_sb = load_w("Wa", AUD, TFD)
        xw_all = wpool.tile([P, N // P, TFD], BF16, tag="xw_all")
        nc.gpsimd.dma_start(
            out=xw_all, in_=xw_ext[:, :].rearrange("(c p) n -> p c n", p=P))
        Wat_sb = load_w("Wat", TFD, D)
        ba_r = load_row("ba", TFD)
        bat_r = load_row("bat", D)
        bk_r = load_row("bk", D)
        bv_r = load_row("bv", D)
        bq_r = load_row("bq", D)
        bo_r = load_row("bo", D)

        def bcast_vec(nm):
            t = wpool.tile([P, D], FP32, tag=nm + "_bc")
            src = wext[nm][:][None, :].broadcast_to([P, D])
            nc.gpsimd.dma_start(out=t, in_=src)
            return t

        gx_bc = bcast_vec("norm_g") if affine_x else None
        bx_bc = bcast_vec("norm_b") if affine_x else None
        gt_bc = bcast_vec("tnorm_g") if affine_t else None
        bt_bc = bcast_vec("tnorm_b") if affine_t else None
        gs_bc = bcast_vec("snorm_g") if affine_s else None
        bs_bc = bcast_vec("snorm_b") if affine_s else None

        # ---------------- shared helpers ----------------
        def ln_stats(src_aps, mv_out):
            stats = work.tile([P, len(src_aps), 6], FP32, tag="stats")
            for j, ap in enumerate(src_aps):
                nc.vector.bn_stats(out=stats[:, j, :], in_=ap)
            nc.vector.bn_aggr(out=mv_out, in_=stats)

        def rstd_sqrt(var_ap, bias_ap):
            # batched into few eras so the sqrt<->exp table switches stay rare
            nc.scalar.activation(out=var_ap, in_=var_ap,
                                 func=AF.Sqrt, bias=bias_ap, scale=1.0)
            nc.vector.reciprocal(out=var_ap, in_=var_ap)

        def ln_apply(src_ap, dst_ap, mean_ap, rstd_ap, g_bc, b_bc, gslc=None):
            if g_bc is None:
                nc.vector.tensor_scalar(
                    out=dst_ap, in0=src_ap, scalar1=mean_ap, scalar2=rstd_ap,
                    op0=OP.subtract, op1=OP.mult)
            else:
                tmpf = work.tile([P, 512], FP32, tag="lnt")
                sl = tmpf[:, 0:src_ap.free_size()]
                nc.vector.tensor_scalar(
                    out=sl, in0=src_ap, scalar1=mean_ap, scalar2=rstd_ap,
                    op0=OP.subtract, op1=OP.mult)
                nc.vector.tensor_mul(out=sl, in0=sl, in1=g_bc[:, gslc])
                nc.vector.tensor_add(out=dst_ap, in0=sl, in1=b_bc[:, gslc])

        def load_xlnT_group(g):
            xlnT = xtp.tile([P, DC, 512], BF16, tag="xlnT")
            for c in range(DC):
                nc.sync.dma_start(out=xlnT[:, c, :],
                                  in_=xln_dram[g][:, c * P:(c + 1) * P],
                                  transpose=True)
            return xlnT

        # ================ n-path + s-path + x-prepass =====================
        with tc.tile_pool(name="nkv", bufs=1) as nkv, \
             tc.tile_pool(name="nsc", bufs=1) as nsc, \
             tc.tile_pool(name="xpre", bufs=4) as xpre, \
             tc.tile_pool(name="xlnst", bufs=2) as xlnst, \
             tc.tile_pool(name="pacc", bufs=2, space="PSUM") as pacc, \
             tc.tile_pool(name="psmall", bufs=2, space="PSUM") as psmall, \
             tc.tile_pool(name="ptp", bufs=2, space="PSUM") as ptp:

            Wk_sb = load_w("Wk", D, D, pool=nkv)
            Wv_sb = load_w("Wv", D, D, pool=nkv)
            # xs as bf16 (cast -> gpsimd)
            xs_sb = nsc.tile([P, ST, D], BF16, tag="xs_sb")
            nc.gpsimd.dma_start(
                out=xs_sb, in_=xs_ext[:, :].rearrange("(c p) n -> p c n", p=P))
            # x tiles as bf16 for stats+LN (cast); interleave Wq/Wo on the
            # same queue so neither starves
            mvall = nsc.tile([P, TT, 2], FP32, tag="mvall")
            xpre_groups = {}

            def load_x_group(g):
                xg = xpre.tile([P, 4, D], BF16, tag="x_pre")
                nc.gpsimd.dma_start(
                    out=xg,
                    in_=x_ext[g * 4 * P:(g + 1) * 4 * P, :]
                    .rearrange("(c p) n -> p c n", p=P))
                xpre_groups[g] = xg

            load_x_group(0)
            load_x_group(1)
            Wq_sb = load_w("Wq", D, D)
            load_x_group(2)
            load_x_group(3)
            Wo_sb = load_w("Wo", D, D)

            def stats_x_group(g):
                xg = xpre_groups[g]
                for j in range(4):
                    ln_stats((xg[:, j, 0:512], xg[:, j, 512:1024]),
                             mvall[:, 4 * g + j, :])

            def rstd_x_era(glo, ghi):
                rstd_sqrt(mvall[:, 4 * glo:4 * ghi, 1], eps_t)

            def apply_store_group(g):
                xg = xpre_groups.pop(g)
                xlng = xlnst.tile([P, 4, D], BF16, tag="xln_t")
                for j in range(4):
                    tt = 4 * g + j
                    if gx_bc is None:
                        nc.vector.tensor_scalar(
                            out=xlng[:, j, :], in0=xg[:, j, :],
                            scalar1=mvall[:, tt, 0:1],
                            scalar2=mvall[:, tt, 1:2],
                            op0=OP.subtract, op1=OP.mult)
                    else:
                        for jj in range(2):
                            ln_apply(xg[:, j, jj * 512:(jj + 1) * 512],
                                     xlng[:, j, jj * 512:(jj + 1) * 512],
                                     mvall[:, tt, 0:1], mvall[:, tt, 1:2],
                                     gx_bc, bx_bc,
                                     slice(jj * 512, (jj + 1) * 512))
                nc.gpsimd.dma_start(
                    out=xln_dram[g][:, :].rearrange("(c p) n -> p c n", p=P),
                    in_=xlng)

            def transpose_to(src_bf, dstT, copy_eng=None):
                nchunk = src_bf.shape[-1] // P
                for g2 in range(0, nchunk, 4):
                    cnt = min(4, nchunk - g2)
                    tps = ptp.tile([P, 512], BF16, tag="tpbf")
                    for k in range(cnt):
                        c = g2 + k
                        nc.tensor.transpose(tps[:, k * P:(k + 1) * P],
                                            src_bf[:, c * P:(c + 1) * P],
                                            ident_bf)
                    src = tps[:, 0:cnt * P].rearrange("p (a b) -> p a b", a=cnt)
                    if copy_eng == "scalar":
                        nc.scalar.copy(out=dstT[:, g2:g2 + cnt, :], in_=src)
                    else:
                        nc.vector.tensor_copy(out=dstT[:, g2:g2 + cnt, :],
                                              in_=src)

            # ---- hsel = per-chunk transpose of hmask ----
            for c in range(DC):
                tphs = psmall.tile([H, P], BF16, tag="small")
                nc.tensor.transpose(tphs, hmask[:, c, :], ident_bf)
                nc.vector.tensor_copy(out=hsel[:, c, :], in_=tphs)

            # ---- s-path stats (xs lands early) ----
            mvs = nsc.tile([P, ST, 2], FP32, tag="mvs")
            for st in range(ST):
                ln_stats((xs_sb[:, st, 0:512], xs_sb[:, st, 512:1024]),
                         mvs[:, st, :])

            # ---- n1: xf_projT [P, 2] f32 ----
            xfpT = nsc.tile([P, 2], FP32, tag="xfpT")
            for m in range(2):
                ps = psmall.tile([P, 1], FP32, tag="small")
                nmm = AUD // P
                for ac in range(nmm):
                    nc.tensor.matmul(ps, lhsT=Wa_sb[:, ac, m * P:(m + 1) * P],
                                     rhs=xf_col[:, ac:ac + 1],
                                     start=(ac == 0),
                                     stop=(ba_r is None and ac == nmm - 1))
                if ba_r is not None:
                    nc.tensor.matmul(ps, lhsT=ba_r[0:1, m * P:(m + 1) * P],
                                     rhs=ones1_bf[0:1, 0:1], start=False,
                                     stop=True)
                nc.vector.tensor_copy(out=xfpT[:, m:m + 1], in_=ps)

            # ---- n2: xcT [P, 2, N] bf16 (xw transposed) ----
            xcT = nsc.tile([P, 2, N], BF16, tag="xcT")
            for nt in range(N // P):
                for tc2 in range(2):
                    tp = ptp.tile([P, P], BF16, tag="tpbf")
                    nc.tensor.transpose(tp,
                                        xw_all[:, nt, tc2 * P:(tc2 + 1) * P],
                                        ident_bf)
                    nc.vector.tensor_copy(out=xcT[:, tc2, nt * P:(nt + 1) * P],
                                          in_=tp)
            xfpT_bf = nsc.tile([P, 2], BF16, tag="xfpT_bf")
            nc.vector.tensor_copy(out=xfpT_bf, in_=xfpT)

            # ---- n3: Wat projections -> SBUF bf16 (raw, pre-LN) ----
            tn_raw = nsc.tile([P, N // P, D], BF16, tag="tn_raw")
            for nt in range(N // P):
                for jh in range(2):
                    ps = pproj.tile([P, 512], FP32, tag="proj")
                    for tc2 in range(2):
                        nc.tensor.matmul(
                            ps, lhsT=xcT[:, tc2, nt * P:(nt + 1) * P],
                            rhs=Wat_sb[:, tc2, jh * 512:(jh + 1) * 512],
                            start=(tc2 == 0),
                            stop=(bat_r is None and tc2 == 1))
                    if bat_r is not None:
                        nc.tensor.matmul(
                            ps, lhsT=ones1_bf,
                            rhs=bat_r[0:1, jh * 512:(jh + 1) * 512],
                            start=False, stop=True)
                    nc.scalar.copy(out=tn_raw[:, nt, jh * 512:(jh + 1) * 512],
                                   in_=ps)
            rep_raw = nsc.tile([1, D], FP32, tag="rep_raw")
            for jh in range(2):
                ps = pproj.tile([1, 512], FP32, tag="proj")
                for tc2 in range(2):
                    nc.tensor.matmul(
                        ps, lhsT=xfpT_bf[:, tc2:tc2 + 1],
                        rhs=Wat_sb[:, tc2, jh * 512:(jh + 1) * 512],
                        start=(tc2 == 0),
                        stop=(bat_r is None and tc2 == 1))
                if bat_r is not None:
                    nc.tensor.matmul(
                        ps, lhsT=ones1_bf[0:1, 0:1],
                        rhs=bat_r[0:1, jh * 512:(jh + 1) * 512],
                        start=False, stop=True)
                nc.scalar.copy(out=rep_raw[0:1, jh * 512:(jh + 1) * 512],
                               in_=ps)

            # ---- n4: batched stats + rstds (all ln/exp, no sqrt) ----
            mvn = nsc.tile([P, N // P, 2], FP32, tag="mvn")
            for nt in range(N // P):
                ln_stats((tn_raw[:, nt, 0:512], tn_raw[:, nt, 512:1024]),
                         mvn[:, nt, :])
            mvr = nsc.tile([1, 2], FP32, tag="mvr")
            statsr = work.tile([1, 2, 6], FP32, tag="statsr")
            nc.vector.bn_stats(out=statsr[0:1, 0, :], in_=rep_raw[0:1, 0:512])
            nc.vector.bn_stats(out=statsr[0:1, 1, :],
                               in_=rep_raw[0:1, 512:1024])
            nc.vector.bn_aggr(out=mvr, in_=statsr)
            rstd_sqrt(mvs[:, :, 1], eps_t)
            rstd_sqrt(mvn[:, :, 1], eps_t)
            rstd_sqrt(mvr[0:1, 1:2], eps_t[0:1, :])

            # ---- K/V + attn/denominator accumulation machinery ----
            def kv_attn_phase(nseq_tiles, recip_dst, make_actT, tail_fn=None,
                              after_tile=None):
                acc0 = pacc.tile([P, 512], FP32, tag="acc")
                acc1 = pacc.tile([P, 512], FP32, tag="acc")
                acc = [acc0, acc1]
                dT = psmall.tile([P, DC], FP32, tag="small")
                nc.vector.memset(acc0, 0.0)
                nc.vector.memset(acc1, 0.0)
                nc.vector.memset(dT, 0.0)
                for it in range(nseq_tiles):
                    actT = make_actT(it)
                    ek = work.tile([P, D], BF16, tag="ek_t")
                    vv = work.tile([P, D], BF16, tag="v_t")
                    for w_sb, b_r, is_k in ((Wk_sb, bk_r, True),
                                            (Wv_sb, bv_r, False)):
                        for jh in range(2):
                            ps = pproj.tile([P, 512], FP32, tag="proj")
                            for c in range(DC):
                                nc.tensor.matmul(
                                    ps, lhsT=actT[:, c, :],
                                    rhs=w_sb[:, c, jh * 512:(jh + 1) * 512],
                                    start=(c == 0),
                                    stop=(b_r is None and c == DC - 1))
                            if b_r is not None:
                                nc.tensor.matmul(
                                    ps, lhsT=ones1_bf,
                                    rhs=b_r[0:1, jh * 512:(jh + 1) * 512],
                                    start=False, stop=True)
                            if is_k:
                                nc.scalar.activation(
                                    out=ek[:, jh * 512:(jh + 1) * 512],
                                    in_=ps, func=AF.Exp)
                            else:
                                nc.scalar.copy(
                                    out=vv[:, jh * 512:(jh + 1) * 512], in_=ps)
                    last = (it == nseq_tiles - 1) and tail_fn is None
                    for c in range(DC):
                        nc.tensor.matmul(
                            acc[c // 4][:, (c % 4) * P:(c % 4 + 1) * P],
                            lhsT=ek[:, c * P:(c + 1) * P],
                            rhs=vv[:, c * P:(c + 1) * P],
                            start=False, stop=last, skip_group_check=True)
                    for c in range(DC):
                        nc.tensor.matmul(
                            dT[:, c:c + 1],
                            lhsT=ek[:, c * P:(c + 1) * P],
                            rhs=onescol_bf,
                            start=False, stop=last, skip_group_check=True)
                    if after_tile is not None:
                        after_tile(it)
                if tail_fn is not None:
                    tail_fn(acc, dT)
                nc.vector.reciprocal(out=recip_dst, in_=dT)
                return acc

            # ---- n5: K/V over the 4 xw tiles ----
            def make_tnT(nt):
                tn_t = work.tile([P, D], BF16, tag="tn_t")
                if gt_bc is None:
                    nc.vector.tensor_scalar(
                        out=tn_t, in0=tn_raw[:, nt, :],
                        scalar1=mvn[:, nt, 0:1], scalar2=mvn[:, nt, 1:2],
                        op0=OP.subtract, op1=OP.mult)
                else:
                    for j in range(2):
                        ln_apply(tn_raw[:, nt, j * 512:(j + 1) * 512],
                                 tn_t[:, j * 512:(j + 1) * 512],
                                 mvn[:, nt, 0:1], mvn[:, nt, 1:2],
                                 gt_bc, bt_bc, slice(j * 512, (j + 1) * 512))
                tnT = work.tile([P, DC, P], BF16, tag="tnT")
                transpose_to(tn_t, tnT)
                return tnT

            # x-prepass chunks interleaved into the n kv loop:
            # stats one group ahead of apply so the DVE apply never waits
            # on the scalar-queue rstd
            def n_after_tile(nt):
                stats_x_group(nt)          # g = 0..3

            # rep-row LN + K/V
            tn_rep = nsc.tile([1, D], BF16, tag="tn_rep")
            if gt_bc is None:
                nc.vector.tensor_scalar(
                    out=tn_rep, in0=rep_raw,
                    scalar1=mvr[0:1, 0:1], scalar2=mvr[0:1, 1:2],
                    op0=OP.subtract, op1=OP.mult)
            else:
                for j in range(2):
                    tmpr = work.tile([1, 512], FP32, tag="tmpr")
                    nc.vector.tensor_scalar(
                        out=tmpr, in0=rep_raw[0:1, j * 512:(j + 1) * 512],
                        scalar1=mvr[0:1, 0:1], scalar2=mvr[0:1, 1:2],
                        op0=OP.subtract, op1=OP.mult)
                    nc.vector.tensor_mul(
                        out=tmpr, in0=tmpr,
                        in1=gt_bc[0:1, j * 512:(j + 1) * 512])
                    nc.vector.tensor_add(
                        out=tn_rep[0:1, j * 512:(j + 1) * 512], in0=tmpr,
                        in1=bt_bc[0:1, j * 512:(j + 1) * 512])
            tpr = ptp.tile([P, DC, 2], BF16, tag="tpbf")
            for c in range(DC):
                nc.tensor.transpose(tpr[:, c, 0:1],
                                    tn_rep[0:1, c * P:(c + 1) * P],
                                    ident_bf[0:1, 0:1])
            tnT_rep = work.tile([P, DC], BF16, tag="tnT_rep")
            nc.vector.tensor_copy(out=tnT_rep[:, :, None], in_=tpr[:, :, 0:1])
            ekr = nsc.tile([1, D], BF16, tag="ekr")
            vrep = nsc.tile([1, D], BF16, tag="vrep")
            for w_sb, b_r, is_k in ((Wk_sb, bk_r, True),
                                    (Wv_sb, bv_r, False)):
                for jh in range(2):
                    ps = pproj.tile([1, 512], FP32, tag="proj")
                    for c in range(DC):
                        nc.tensor.matmul(
                            ps, lhsT=tnT_rep[:, c:c + 1],
                            rhs=w_sb[:, c, jh * 512:(jh + 1) * 512],
                            start=(c == 0),
                            stop=(b_r is None and c == DC - 1))
                    if b_r is not None:
                        nc.tensor.matmul(
                            ps, lhsT=ones1_bf[0:1, 0:1],
                            rhs=b_r[0:1, jh * 512:(jh + 1) * 512],
                            start=False, stop=True)
                    if is_k:
                        nc.scalar.activation(
                            out=ekr[0:1, jh * 512:(jh + 1) * 512],
                            in_=ps, func=AF.Exp, bias=ln512_t[0:1, :])
                    else:
                        nc.scalar.copy(
                            out=vrep[0:1, jh * 512:(jh + 1) * 512], in_=ps)

            def rep_tail(acc, dT):
                for c in range(DC):
                    nc.tensor.matmul(
                        acc[c // 4][:, (c % 4) * P:(c % 4 + 1) * P],
                        lhsT=ekr[0:1, c * P:(c + 1) * P],
                        rhs=vrep[0:1, c * P:(c + 1) * P],
                        start=False, stop=True, skip_group_check=True)
                for c in range(DC):
                    nc.tensor.matmul(
                        dT[:, c:c + 1],
                        lhsT=ekr[0:1, c * P:(c + 1) * P],
                        rhs=ones1_bf[0:1, 0:1],
                        start=False, stop=True, skip_group_check=True)

            recipTk = npool.tile([P, DC], FP32, tag="recipTk")
            acc_k = kv_attn_phase(N // P, recipTk, make_tnT,
                                  tail_fn=rep_tail, after_tile=n_after_tile)

            # scale attn rows by recip_k -> SBUF f32 (drains acc_k)
            attn_sc = nsc.tile([P, DC, P], FP32, tag="attn_sc")
            for c in range(DC):
                nc.vector.tensor_scalar_mul(
                    out=attn_sc[:, c, :],
                    in0=acc_k[c // 4][:, (c % 4) * P:(c % 4 + 1) * P],
                    scalar1=recipTk[:, c:c + 1])

            # sqrt era 2: x groups 0-3, then their LN applies + stores
            rstd_x_era(0, 4)
            for g in range(4):
                apply_store_group(g)
            for g in range(4, NG):
                load_x_group(g)
            xlnT_tiles = {0: load_xlnT_group(0), 1: load_xlnT_group(1)}

            # ---- s-path K/V, with remaining prepass chunks interleaved ---
            def make_snT(st):
                sn_t = work.tile([P, D], BF16, tag="tn_t")
                if gs_bc is None:
                    nc.vector.tensor_scalar(
                        out=sn_t, in0=xs_sb[:, st, :], scalar1=mvs[:, st, 0:1],
                        scalar2=mvs[:, st, 1:2], op0=OP.subtract, op1=OP.mult)
                else:
                    for j in range(2):
                        ln_apply(xs_sb[:, st, j * 512:(j + 1) * 512],
                                 sn_t[:, j * 512:(j + 1) * 512],
                                 mvs[:, st, 0:1], mvs[:, st, 1:2],
                                 gs_bc, bs_bc, slice(j * 512, (j + 1) * 512))
                snT = work.tile([P, DC, P], BF16, tag="tnT")
                transpose_to(sn_t, snT)
                return snT

            def s_after_tile(st):
                stats_x_group(4 + st)      # g = 4..7

            recipTs = npool.tile([P, DC], FP32, tag="recipTs")
            acc_s = kv_attn_phase(ST, recipTs, make_snT,
                                  after_tile=s_after_tile)
            # sqrt era 3: x groups 4-7, then their LN applies + stores
            rstd_x_era(4, NG)
            for g in range(4, NG):
                apply_store_group(g)

            # sattnsum rows: scale G rows by recip_s, mask, column-sum
            sattn_row = nsc.tile([1, DC, P], FP32, tag="sattn_row")
            for c in range(DC):
                gsc = work.tile([P, P], FP32, tag="gsc")
                nc.vector.tensor_scalar_mul(
                    out=gsc, in0=acc_s[c // 4][:, (c % 4) * P:(c % 4 + 1) * P],
                    scalar1=recipTs[:, c:c + 1])
                nc.vector.tensor_mul(out=gsc, in0=gsc, in1=mask_f)
                ssp = psmall.tile([1, P], FP32, tag="small")
                nc.tensor.matmul(ssp, lhsT=onescol_f, rhs=gsc,
                                 start=True, stop=True)
                nc.vector.tensor_copy(out=sattn_row[0:1, c, :], in_=ssp)

            # ---- attn2 block-diagonal tiles ----
            attn2 = npool.tile([P, DC, P], BF16, tag="attn2")
            for c in range(DC):
                psb = psmall.tile([P, P], FP32, tag="small")
                nc.tensor.matmul(psb, lhsT=ones1_f, rhs=sattn_row[0:1, c, :],
                                 start=True, stop=True)
                tmp = work.tile([P, P], FP32, tag="a2tmp")
                nc.vector.tensor_add(out=tmp, in0=attn_sc[:, c, :], in1=psb)
                nc.vector.tensor_mul(out=attn2[:, c, :], in0=tmp, in1=mask_f)


        # ================ x-phase: feature-major q + attn2 + out ==========
        with tc.tile_pool(name="xeq", bufs=4) as xeq, \
             tc.tile_pool(name="xres", bufs=3) as xres, \
             tc.tile_pool(name="chain", bufs=2) as chain, \
             tc.tile_pool(name="pdsum", bufs=2, space="PSUM") as pdsum, \
             tc.tile_pool(name="prep", bufs=2, space="PSUM") as prep, \
             tc.tile_pool(name="pyT", bufs=2, space="PSUM") as pyT:

            # A-phase split into pipelineable parts
            def a_qproj(g, xlnT):
                Eq = xeq.tile([P, DC, 512], BF16, tag="Eq")
                for c in range(DC):
                    ps = pproj.tile([P, 512], FP32, tag="proj")
                    for kc in range(DC):
                        nc.tensor.matmul(
                            ps, lhsT=Wq_sb[:, kc, c * P:(c + 1) * P],
                            rhs=xlnT[:, kc, :],
                            start=(kc == 0),
                            stop=(bq_r is None and kc == DC - 1))
                    if bq_r is not None:
                        nc.tensor.matmul(
                            ps, lhsT=bq_r[0:1, c * P:(c + 1) * P],
                            rhs=ones512_bf,
                            start=False, stop=True)
                    nc.scalar.activation(out=Eq[:, c, :], in_=ps, func=AF.Exp)
                return Eq

            def a_dsum(g, Eq):
                psd = pdsum.tile([H, 512], FP32, tag="dsum")
                for c in range(DC):
                    nc.tensor.matmul(psd, lhsT=hmask[:, c, :],
                                     rhs=Eq[:, c, :],
                                     start=(c == 0), stop=(c == DC - 1))
                recip16 = work.tile([H, 512], BF16, tag="recip16")
                with nc.allow_low_precision(
                        reason="softmax denom recip in bf16; ~0.4% rel "
                               "on q rows, well inside the 2e-2 budget"):
                    nc.vector.reciprocal(out=recip16, in_=psd)
                return recip16

            def a_rep(g, Eq, recip16):
                for c in range(DC):
                    pr = prep.tile([P, 512], FP32, tag="rep")
                    nc.tensor.matmul(pr, lhsT=hsel[:, c, :], rhs=recip16,
                                     start=True, stop=True)
                    nc.vector.tensor_mul(out=Eq[:, c, :], in0=Eq[:, c, :],
                                         in1=pr)

            def b_attn2(g, Eq):
                for c in range(DC):
                    py = pyT.tile([P, 512], FP32, tag="yT")
                    nc.tensor.matmul(py, lhsT=attn2[:, c, :], rhs=Eq[:, c, :],
                                     start=True, stop=True)
                    nc.scalar.activation(out=Eq[:, c, :], in_=py,
                                         func=AF.Silu)

            def b_outproj(g, siluT):
                for sub in range(4):
                    tt = 4 * g + sub
                    x_res = xres.tile([P, D], FP32, tag="x_res")
                    nc.sync.dma_start(out=x_res,
                                      in_=x_ext[tt * P:(tt + 1) * P, :])
                    o_sb = chain.tile([P, D], FP32, tag="o_sb")
                    for jh in range(2):
                        ps = pproj.tile([P, 512], FP32, tag="proj")
                        for c in range(DC):
                            nc.tensor.matmul(
                                ps,
                                lhsT=siluT[:, c, sub * P:(sub + 1) * P],
                                rhs=Wo_sb[:, c, jh * 512:(jh + 1) * 512],
                                start=(c == 0),
                                stop=(bo_r is None and c == DC - 1))
                        if bo_r is not None:
                            nc.tensor.matmul(
                                ps, lhsT=ones1_bf,
                                rhs=bo_r[0:1, jh * 512:(jh + 1) * 512],
                                start=False, stop=True)
                        nc.vector.tensor_add(
                            out=o_sb[:, jh * 512:(jh + 1) * 512], in0=ps,
                            in1=x_res[:, jh * 512:(jh + 1) * 512])
                    nc.sync.dma_start(out=out_ext[tt * P:(tt + 1) * P, :],
                                      in_=o_sb)

            # superbatches of SB groups: [A x SB][B x SB]; A stages
            # software-pipelined so dsum/rep latencies hide under the next
            # group's q-projection
            for sb in range(NG // SB):
                gs = list(range(sb * SB, (sb + 1) * SB))
                eqs = {}
                pend = []          # (g, Eq) awaiting dsum+rep
                for g in gs:
                    if g not in xlnT_tiles:
                        xlnT_tiles[g] = load_xlnT_group(g)
                    if g + 2 < NG and (g + 2) not in xlnT_tiles:
                        xlnT_tiles[g + 2] = load_xlnT_group(g + 2)
                    eqs[g] = a_qproj(g, xlnT_tiles.pop(g))
                    if pend:
                        pg, pEq, prc = pend.pop(0)
                        a_rep(pg, pEq, prc)
                    pend.append((g, eqs[g], a_dsum(g, eqs[g])))
                while pend:
                    pg, pEq, prc = pend.pop(0)
                    a_rep(pg, pEq, prc)
                for g in gs:
                    b_attn2(g, eqs[g])
                for g in gs:
                    b_outproj(g, eqs[g])

    nc.compile()
    return nc


def kernel(**inputs) -> np.ndarray:
    from concourse.bass_utils import run_bass_kernel_spmd

    ins = {k: np.ascontiguousarray(np.asarray(v, dtype=np.float32))
           for k, v in inputs.items()}
    affine_x = not (np.all(ins["norm_g"] == 1.0) and np.all(ins["norm_b"] == 0.0))
    affine_t = not (np.all(ins["tnorm_g"] == 1.0) and np.all(ins["tnorm_b"] == 0.0))
    affine_s = not (np.all(ins["snorm_g"] == 1.0) and np.all(ins["snorm_b"] == 0.0))
    hasb = {nm: bool(np.any(ins[nm] != 0.0))
            for nm in ("bq", "bk", "bv", "ba", "bat", "bo")}

    key = (affine_x, affine_t, affine_s, tuple(sorted(hasb.items())))
    if key not in _CACHE:
        _CACHE[key] = _build(affine_x, affine_t, affine_s, hasb)
    nc = _CACHE[key]

    wnames = ["norm_g", "norm_b", "tnorm_g", "tnorm_b", "snorm_g", "snorm_b",
              "Wq", "bq", "Wk", "bk", "Wv", "bv", "Wa", "ba", "Wat", "bat",
              "Wo", "bo"]
    in_maps = []
    for b in range(NCORES):
        m = {"x": ins["x"][b], "xf": ins["xf"][b], "xw": ins["xw"][b],
             "xs": ins["xs"][b]}
        for nm in wnames:
            m[nm] = ins[nm]
        in_maps.append(m)

    res = run_bass_kernel_spmd(nc, in_maps, core_ids=list(range(NCORES)))
    return np.stack([res.results[i]["out"] for i in range(NCORES)], axis=0)


if __name__ == "__main__":
    import reference
    rin = reference.setup_inputs()
    out = kernel(**{k: np.asarray(v) for k, v in rin.items()})
    print("out shape:", out.shape, out.dtype)
